# revision 11
# baseline (speedup 1.0000x reference)
"""GroupSort (k=4) Trainium2 Bass kernel, v5.

x: (16384, 4096) f32. Sort each contiguous group of 4 along the last dim.
Sharding: batch-parallel across 8 NeuronCores (2048 rows/core), no comms.

Measured HW rules (microbench.py / microbench2.py, v4 trace):
  * DVE TensorTensor = 2x (0.6ns/elem) when operand reads are unit runs /
    run-of-2 / long 2-block views AND writes are sequential-ish blocks;
    single-elem strided reads or alternating-block writes = 1x-0.25x.
    scalar_tensor_tensor never exceeds 1x. DVE COPY = 4x on unit.
  * ACT gather (stride-4 single read -> unit write) ~1.0-1.2ns/elem.
  * DMA: 16 engines, ~325 GB/s/core effective; load+store 32 MiB/core
    is the traffic floor (bf16 in+out; fp8 fails the 2e-2 rel-err gate).

v5 = v4 + variable tile schedule. v4 was DVE-bound (6 TT x 1.2us/tile,
zero gaps) with an 18us ramp (full 1MiB tile-0 load + gathers before the
first TT). Small leading segments (1024/1024/2048/4096 elems/partition)
prime the pipeline ~7us earlier; 8192-wide steady tiles halve per-op
overheads and double DMA chunk size to 16KB/partition.

Pipeline per segment [128p, Fs free], Gs = Fs/4:
  SP    load segment -> tin slot                  (HWDGE, bf16)
  ACT   4 lane-gathers  tin[(g k)] -> ln [e0|e1|e2|e3]  (~1.05ns/elem)
        + issues store of segment i-1 from w slot
  DVE   network (0,1),(2,3) / (0,2),(1,3) / (1,2), 6 TTs all 2x shapes:
          s1 min/max([e0|e2],[e1|e3]) -> s1=[p|q|P|Q] lanes
          s2 min/max([p|P],[q|Q])     -> w: l0->1, w1->5 / w0->0, l3->4
          s3 min/max(w0, w1)          -> l1->2, l2->3
        w slot layout [w0|l0|l1|l2|l3|w1]: all writes are increasing
        uniform 2-blocks; final lanes contiguous in-order at w[G:5G].
  Host  casts x->bf16 before upload (monotone rounding: identical rel
        err to v2's bf16 output path) and re-interleaves lanes during
        the unshard (transpose of [..., 4, G] axes).
"""

import numpy as np

B, D, K = 16384, 4096, 4
NCORES = 8
RPC = B // NCORES  # rows per core
N = RPC * D  # flat elements per core
P = 128  # SBUF partitions
PPF = N // P  # free elems per partition per core (65536)
FMAX = 8192
# ramp: small head segments prime the pipeline (DVE start ~11us vs 18);
# two 4096s build ACT's lead before the 8192 steady tiles; small tail
# segment keeps the last (compute-gated) store short.
SEGS = [1024, 1024, 2048, 2048, 4096, 4096] + [8192] * 6 + [2048]
assert sum(SEGS) == PPF
NSEG = len(SEGS)
NBUF = 3  # tin slots (FMAX each)
NBUF_L = 2  # lane-buffer slots
NBUF_W = 3  # output slots
ORDERED_LOADS = False  # sim-only: CoreSim's sem checker rejects the
# no-completion-ordering-wait HWDGE pattern (safe on HW: FIFO drain)

_cache = {}


def _ap(t, offset, dims):
    """Raw AP over SBUF tensor t: partition dim + given [stride, count] dims."""
    from concourse.ap import AP

    base = t[:]
    return AP(base.tensor, offset, [list(base.ap[0])] + [list(d) for d in dims])


def _build():
    import concourse.bass as bass
    import concourse.mybir as mybir

    bf16 = mybir.dt.bfloat16
    mn = mybir.AluOpType.min
    mx = mybir.AluOpType.max

    nc = bass.Bass()
    x = nc.dram_tensor("x", [N], bf16, kind="ExternalInput")
    y = nc.dram_tensor("y", [N], bf16, kind="ExternalOutput")
    # each segment i is the contiguous flat chunk [P*off_i, P*(off_i+Fs_i)),
    # viewed [P, Fs]: partition p holds flat[P*off + p*Fs : ... + Fs].
    # Groups of 4 never straddle partitions (off, Fs multiples of 1024).
    seg_off = []
    o = 0
    for fs in SEGS:
        seg_off.append(o)
        o += fs

    with (
        nc.sbuf_tensor([P, NBUF * FMAX], bf16) as tin,
        nc.sbuf_tensor([P, NBUF_L * FMAX], bf16) as ln,
        nc.sbuf_tensor([P, FMAX], bf16) as s1,
        nc.sbuf_tensor([P, NBUF_W * 6 * (FMAX // K)], bf16) as w,
        nc.semaphore("dma_in") as dma_in,
        nc.semaphore("dma_out") as dma_out,
        nc.semaphore("s_act") as s_act,
        nc.semaphore("s_net") as s_net,
        nc.Block() as block,
    ):
        GW = FMAX // K  # w sub-slot lane capacity

        def x_seg(i):
            fs = SEGS[i]
            return x[P * seg_off[i] : P * (seg_off[i] + fs)].rearrange(
                "(p f) -> p f", p=P
            )

        def y_seg(i):
            fs = SEGS[i]
            return y[P * seg_off[i] : P * (seg_off[i] + fs)].rearrange(
                "(p f) -> p f", p=P
            )

        @block.sync
        def _(sync):
            for i in range(NSEG):
                if ORDERED_LOADS and i > 0:
                    sync.wait_ge(dma_in, 16 * i)
                if i >= NBUF:
                    sync.wait_ge(s_act, i - NBUF + 1)
                s = i % NBUF
                sync.dma_start(
                    tin[:, s * FMAX : s * FMAX + SEGS[i]], x_seg(i)
                ).then_inc(dma_in, 16)

        @block.scalar
        def _(scalar):
            for i in range(NSEG + 1):
                if i < NSEG:
                    fs = SEGS[i]
                    gs = fs // K
                    s = i % NBUF
                    sl = i % NBUF_L
                    scalar.wait_ge(dma_in, 16 * (i + 1))
                    if i >= NBUF_L:
                        scalar.wait_ge(s_net, i - NBUF_L + 1)
                    tin4 = tin[:, s * FMAX : s * FMAX + fs].rearrange(
                        "p (g k) -> p g k", k=K
                    )
                    for j in range(K):
                        scalar.copy(
                            ln[:, sl * FMAX + j * gs : sl * FMAX + (j + 1) * gs],
                            tin4[:, :, j],
                        )
                    scalar.drain().then_inc(s_act, 1)
                j = i - 1  # issue store for the previous segment
                if 0 <= j < NSEG:
                    gj = SEGS[j] // K
                    sw = j % NBUF_W
                    scalar.wait_ge(s_net, j + 1)
                    scalar.dma_start(
                        y_seg(j),
                        w[:, sw * 6 * GW + gj : sw * 6 * GW + 5 * gj],
                    ).then_inc(dma_out, 16)

        @block.vector
        def _(vector):
            for i in range(NSEG):
                fs = SEGS[i]
                gs = fs // K
                sl = i % NBUF_L
                sw = i % NBUF_W
                vector.wait_ge(s_act, i + 1)
                if i >= NBUF_W:
                    vector.wait_ge(dma_out, 16 * (i - NBUF_W + 1))
                lb = sl * FMAX  # lane base: [e0|e1|e2|e3] each gs wide
                A1 = _ap(ln, lb, [[2 * gs, 2], [1, gs]])  # [e0 | e2]
                B1 = _ap(ln, lb + gs, [[2 * gs, 2], [1, gs]])  # [e1 | e3]
                # s1 = [p | q | P | Q] lanes, each gs
                s1m = _ap(s1, 0, [[gs, 2], [1, gs]])  # p->0, q->1
                s1x = _ap(s1, 2 * gs, [[gs, 2], [1, gs]])  # P->2, Q->3
                vector.tensor_tensor(s1m, A1, B1, mn)
                vector.tensor_tensor(s1x, A1, B1, mx)
                A2 = _ap(s1, 0, [[2 * gs, 2], [1, gs]])  # [p | P]
                B2 = _ap(s1, gs, [[2 * gs, 2], [1, gs]])  # [q | Q]
                # w slots: [w0 | l0 | l1 | l2 | l3 | w1], each gs wide
                off = sw * 6 * GW
                dmin = _ap(w, off + gs, [[4 * gs, 2], [1, gs]])  # l0->1, w1->5
                dmax = _ap(w, off, [[4 * gs, 2], [1, gs]])  # w0->0, l3->4
                vector.tensor_tensor(dmin, A2, B2, mn)
                vector.tensor_tensor(dmax, A2, B2, mx)
                w0v = w[:, off : off + gs]
                w1v = w[:, off + 5 * gs : off + 6 * gs]
                vector.tensor_tensor(w[:, off + 2 * gs : off + 3 * gs], w0v, w1v, mn)
                vector.tensor_tensor(w[:, off + 3 * gs : off + 4 * gs], w0v, w1v, mx)
                vector.drain().then_inc(s_net, 1)

    return nc


def _run(x_np, trace=False, trace_kwargs=None):
    import ml_dtypes
    from concourse.bass_utils import run_bass_kernel_spmd

    if "nc" not in _cache:
        _cache["nc"] = _build()
    nc = _cache["nc"]

    xb = np.ascontiguousarray(x_np).astype(ml_dtypes.bfloat16)
    shards = np.split(xb, NCORES, axis=0)
    in_maps = [{"x": s.reshape(-1)} for s in shards]
    res = run_bass_kernel_spmd(
        nc,
        in_maps,
        list(range(NCORES)),
        trace=trace,
        **(trace_kwargs or {}),
    )
    outs = []
    for r in res.results:
        yc = np.asarray(r["y"]).reshape(P * PPF)
        parts = []
        o = 0
        for fs in SEGS:
            seg = yc[P * o : P * (o + fs)].reshape(P, K, fs // K)
            parts.append(seg.transpose(0, 2, 1).reshape(-1))
            o += fs
        core = np.concatenate(parts).astype(np.float32)
        outs.append(core.reshape(RPC, D))
    out = np.concatenate(outs, axis=0)
    return out, res


def kernel(x, k):
    assert int(k) == K, f"kernel hardcodes k={K}, got {k}"
    out, _ = _run(np.asarray(x))
    return out
